# revision 44
# baseline (speedup 1.0000x reference)
"""Trainium2 Bass kernel for nn_MultiHeadAttnCoupling.

Reference computation (B=4, N=128, D=32768, heads=8, seq=64, d_tensor=64):
    Q = (z @ Wq + bq).reshape(B,N,H,S,DT)   # per (b,n): attention over S
    K = (x @ Wk + bk).reshape(...)
    V = (x @ Wv + bv).reshape(...)
    out = softmax(Q K^T / 8) V  -> reshape -> @ Wo + bo

Sharding: head-parallel over 8 cores (one head per core); host sums the 8
partial outputs and adds bo.

v2 design ("token pairing"):
  - single 512-token pass; each weight chunk DMA'd once; N=512 proj matmuls.
  - attention processes TWO tokens per matmul via block-diagonal stationaries:
    the 128x128 stationary holds token 2j's K (rows/cols 0-63) and token
    2j+1's K (rows/cols 64-127) with zero off-diagonal blocks (memset once);
    streaming operand stacks the two tokens' q along partitions.  Halves the
    LDWEIGHTS count (the measured bottleneck: ~130ns/LDW regardless of size).
  - all stacked slabs are stored pair-index-innermost so projection evictions
    write contiguous runs (strided writes measured 4-5x slower; strided reads
    are free).  Eviction is 4 ops per chunk (one per src-half x token-parity).
  - attn@V streams V (+ ones column -> softmax denominators in output col 64)
    against the exp'd-scores block-diagonal stationary.
  - output otp is stored in (parity, pair) token order; the host permutes the
    final columns back.
"""

import numpy as np
import ml_dtypes

B, N = 4, 128
INPUT_SIZE, Z_SIZE = 512, 256
DT, H, S = 64, 8, 64
D = DT * H * S            # 32768
DH = S * DT               # 4096 per head
T = B * N                 # 512 tokens
J = T // 2                # 256 token pairs
CH = DH // 128            # 32 chunks per projection
KCQ = Z_SIZE // 128       # 2
KCX = INPUT_SIZE // 128   # 4
CT = INPUT_SIZE // 128    # 4 output col tiles
G = 7                     # pairs per attention group (PSUM bank width)
NW = 3                    # exp'd-scores window ring depth
CB = 4                    # chunks per weight DMA batch
JI = 8                    # kbd pair-block: stationary cols at 16B stride
DB = 6                    # attention groups per otp-rearrange DMA batch

_bf16 = ml_dtypes.bfloat16

_cache = {}


def _build_nc(reps=1):
    import concourse.mybir as mybir
    import concourse.tile as tile
    from concourse import bacc

    f32, bf16 = mybir.dt.float32, mybir.dt.bfloat16
    AF = mybir.ActivationFunctionType
    MUL = mybir.AluOpType.mult

    nc = bacc.Bacc("TRN2", target_bir_lowering=False, debug=False)

    zt_d = nc.dram_tensor("zt", [128, KCQ, T], bf16, kind="ExternalInput")
    xt_d = nc.dram_tensor("xt", [128, KCX, T], bf16, kind="ExternalInput")
    wq_d = nc.dram_tensor("wq", [128, CH, KCQ * 128], bf16, kind="ExternalInput")
    wk_d = nc.dram_tensor("wk", [128, CH, KCX * 128], bf16, kind="ExternalInput")
    wv_d = nc.dram_tensor("wv", [128, CH, KCX * 128], bf16, kind="ExternalInput")
    wo_d = nc.dram_tensor("wo", [128, CT, CH, 128], bf16, kind="ExternalInput")
    bq_d = nc.dram_tensor("bq", [128, CH], f32, kind="ExternalInput")
    bk_d = nc.dram_tensor("bk", [128, CH], f32, kind="ExternalInput")
    bv_d = nc.dram_tensor("bv", [128, CH], f32, kind="ExternalInput")
    pt_d = nc.dram_tensor("pt", [INPUT_SIZE, T], f32, kind="ExternalOutput")

    with tile.TileContext(nc) as tc:
        with (
            tc.tile_pool(name="acts", bufs=1) as acts_pool,
            tc.tile_pool(name="slabs", bufs=1) as slab_pool,
            tc.tile_pool(name="wts", bufs=3) as wts_pool,
            tc.tile_pool(name="rds", bufs=4) as rds_pool,
            tc.tile_pool(name="osb", bufs=2) as osb_pool,
            tc.tile_pool(name="psum", bufs=8, space="PSUM") as psum_pool,
        ):
            # resident activations and biases (z/bq first so Q starts early;
            # x/bk/bv stream in behind the Q projection)
            zt = acts_pool.tile([128, KCQ, T], bf16, tag="zt")
            xt = acts_pool.tile([128, KCX, T], bf16, tag="xt")
            bq = acts_pool.tile([128, CH], f32, tag="bq")
            bk = acts_pool.tile([128, CH], f32, tag="bk")
            bv = acts_pool.tile([128, CH], f32, tag="bv")
            nc.sync.dma_start(zt[:], zt_d[:])
            nc.sync.dma_start(bq[:], bq_d[:])
            nc.sync.dma_start(xt[:], xt_d[:])
            nc.sync.dma_start(bk[:], bk_d[:])
            nc.sync.dma_start(bv[:], bv_d[:])

            for rep in range(reps):
                # stacked slabs, pair index j innermost.
                # kbd[dt+64a, jb, s_k+64a', ji] block-diag (a==a' data, else
                # 0), pair j = jb*JI+ji blocked so LDW cols sit at 16B stride
                # qst[dt+64a, s_q, j]   vst[s_k+64a, dt|ones, j]
                kbd = slab_pool.tile([128, J // JI, 128, JI], bf16, tag="kbd")
                qst = slab_pool.tile([128, S, J], bf16, tag="qst")
                vst = slab_pool.tile([128, DT + 1, J], bf16, tag="vst")
                # exp'd scores ring: ebd[s_k+64a, w, jj, s_q+64a] block-diag
                ebd = slab_pool.tile([128, NW, G, 128], bf16, tag="ebd")
                otp = slab_pool.tile([128, T, CH], bf16, tag="otp")
                # normalized attn@V ring [s_q+64a, ring-pair, dt]; DMA
                # regroups it into otp (partition = s + 64*(dt//32))
                ot2 = slab_pool.tile([128, 2 * DB * G, DT], bf16, tag="ot2")
                # zero the off-diagonal quadrants once (gpsimd is idle)
                nc.gpsimd.memset(kbd[0:64, :, 64:128, :], 0.0)
                nc.gpsimd.memset(kbd[64:128, :, 0:64, :], 0.0)
                nc.gpsimd.memset(ebd[0:64, :, :, 64:128], 0.0)
                nc.gpsimd.memset(ebd[64:128, :, :, 0:64], 0.0)
                nc.vector.memset(vst[:, DT, :], 1.0)

                # ---- projections ----
                # dest quadrant per (src half lo/hi, token parity al):
                #   Q chunk c (s-major): [dt+64al, s_q=2c+hi, j]
                #   K chunk c (s-major): [dt+64al, (s_k=2c+hi)+64al, j]
                #   V chunk c (dt-major): [s_k+64al, dt=2c+hi, j]
                for (w_d, wtag, nkc, act, bias, kind) in (
                    (wq_d, "wq", KCQ, zt, bq, "q"),
                    (wk_d, "wk", KCX, xt, bk, "k"),
                    (wv_d, "wv", KCX, xt, bv, "v"),
                ):
                    for c4 in range(CH // CB):
                        wt = wts_pool.tile([128, CB, nkc, 128], bf16, tag="wt")
                        nc.sync.dma_start(
                            wt[:], w_d[:, CB * c4:CB * (c4 + 1), :].rearrange(
                                "p c (kc m) -> p c kc m", m=128))
                        for ci in range(CB):
                            c = CB * c4 + ci
                            ps = psum_pool.tile([128, T], f32, tag="big",
                                                name=f"pj{rep}{wtag}{c}")
                            for kc in range(nkc):
                                nc.tensor.matmul(
                                    ps[:], wt[:, ci, kc, :], act[:, kc, :],
                                    start=(kc == 0), stop=(kc == nkc - 1))
                            # activations are host-permuted to (parity, pair)
                            # token order: even tokens = cols 0:J, odd = J:2J
                            for hi in range(2):
                                src_lo = ps[64 * hi:64 * hi + 64, 0:J]
                                src_hi = ps[64 * hi:64 * hi + 64, J:T]
                                bia = (None if bias is None else
                                       bias[64 * hi:64 * hi + 64, c:c + 1])
                                m = 2 * c + hi
                                if kind == "q":
                                    # even tokens on ACT, odd tokens on DVE
                                    nc.scalar.activation(
                                        qst[0:64, m, :], src_lo,
                                        AF.Identity, bias=bia)
                                    nc.vector.tensor_scalar_add(
                                        qst[64:128, m, :], src_hi, bia)
                                elif kind == "k":
                                    blk = lambda s: s.rearrange(
                                        "p (a b) -> p a b", b=JI)
                                    nc.scalar.activation(
                                        kbd[0:64, :, m, :], blk(src_lo),
                                        AF.Identity, bias=bia)
                                    nc.vector.tensor_scalar_add(
                                        kbd[64:128, :, 64 + m, :],
                                        blk(src_hi), bia)
                                else:
                                    nc.scalar.activation(
                                        vst[0:64, m, :], src_lo,
                                        AF.Identity, bias=bia)
                                    nc.vector.tensor_scalar_add(
                                        vst[64:128, m, :], src_hi, bia)

                # ---- attention, groups of G pairs ----
                # prefetch all Wo weights now; they land during attention
                # (the Sync queue would otherwise park them behind the last
                # otp drain)
                HC = CH // 2
                wo_tiles = {}
                for ct in range(1):
                    for h2 in range(2):
                        wt = slab_pool.tile([128, HC, 128], bf16,
                                            tag=f"wo{ct}{h2}")
                        nc.sync.dma_start(
                            wt[:], wo_d[:, ct, HC * h2:HC * (h2 + 1), :])
                        wo_tiles[2 * ct + h2] = wt

                # software-pipelined: scores(g+1) issues before attnV(g) so
                # the exp(g) latency on ACT hides under scores(g+1) on PE
                ng = (J + G - 1) // G

                def scores_group(gi):
                    j0 = gi * G
                    g = min(G, J - j0)
                    w = gi % NW
                    sc = psum_pool.tile([128, G, S], f32, tag="big",
                                        name=f"sc{rep}_{gi}")
                    for i in range(g):
                        j = j0 + i
                        nc.tensor.matmul(sc[:, i, :],
                                         kbd[:, j // JI, :, j % JI],
                                         qst[:, :, j],
                                         start=True, stop=True)
                    # exp into block-diag quadrants of the ring window
                    nc.scalar.activation(ebd[0:64, w, 0:g, 0:64],
                                         sc[0:64, 0:g, :], AF.Exp)
                    nc.scalar.activation(ebd[64:128, w, 0:g, 64:128],
                                         sc[64:128, 0:g, :], AF.Exp)

                def attnv_group(gi):
                    j0 = gi * G
                    g = min(G, J - j0)
                    w = gi % NW
                    op = psum_pool.tile([128, G, DT + 1], f32, tag="big",
                                        name=f"op{rep}_{gi}")
                    for i in range(g):
                        nc.tensor.matmul(op[:, i, :], ebd[:, w, i, :],
                                         vst[:, :, j0 + i],
                                         start=True, stop=True)
                    rd = rds_pool.tile([128, G], f32, tag="rd")
                    nc.vector.reciprocal(rd[:, 0:g], op[:, 0:g, DT])
                    # normalize into the ot2 ring (one full-width op)
                    r0 = (gi % (2 * DB)) * G
                    rdb = rd[:, 0:g].unsqueeze(2).broadcast_to([128, g, DT])
                    nc.vector.tensor_tensor(
                        ot2[:, r0:r0 + g, :],
                        op[:, 0:g, 0:DT], rdb, MUL)

                def drain_batch(b):
                    # DMA ot2 ring half -> otp[s+64*(dt//32), al*J+j, dt%32]
                    g0 = b * DB
                    g1 = min(ng, g0 + DB)
                    jb0, jb1 = g0 * G, min(J, g1 * G)
                    r0 = (g0 % (2 * DB)) * G
                    rn = jb1 - jb0
                    for al in range(2):
                        for dh in range(2):
                            nc.sync.dma_start(
                                otp[64 * dh:64 * dh + 64,
                                    al * J + jb0:al * J + jb1, :],
                                ot2[64 * al:64 * al + 64, r0:r0 + rn,
                                    32 * dh:32 * dh + 32])

                nb = (ng + DB - 1) // DB
                for gi in range(ng):
                    scores_group(gi)
                    if gi >= 1:
                        attnv_group(gi - 1)
                        if gi % DB == 0:
                            drain_batch(gi // DB - 1)
                attnv_group(ng - 1)
                for b in range(max(0, (ng - 1) // DB), nb):
                    drain_batch(b)

                # ---- output projection (ct-outer; 2 fins in flight) ----
                for ct in range(CT):
                    fin = psum_pool.tile([128, T], f32, tag="big",
                                         name=f"fin{rep}_{ct}")
                    for h2 in range(2):
                        if 2 * ct + h2 in wo_tiles:
                            wt = wo_tiles[2 * ct + h2]
                        else:
                            wt = wts_pool.tile([128, HC, 128], bf16, tag="wt")
                            nc.sync.dma_start(
                                wt[:], wo_d[:, ct, HC * h2:HC * (h2 + 1), :])
                        for i in range(HC):
                            cc = HC * h2 + i
                            nc.tensor.matmul(
                                fin[:], wt[:, i, :], otp[:, :, cc],
                                start=(cc == 0), stop=(cc == CH - 1))
                    ob = osb_pool.tile([128, T], f32, tag="ob")
                    nc.vector.tensor_copy(ob[:], fin[:])
                    nc.sync.dma_start(pt_d[128 * ct:128 * (ct + 1), :], ob[:])

    nc.compile()
    return nc


# dt-major permutation: new index dt*S+s  <- old index s*DT+dt
_PERM = np.arange(S * DT).reshape(S, DT).T.reshape(-1)
# kernel processes tokens in (parity, pair) order: position t' holds token
# TOK[t'];  _TPERM[t] = position of token t (inverse)
_TOK = np.concatenate([np.arange(0, T, 2), np.arange(1, T, 2)])
_TPERM = (np.arange(T) % 2) * J + np.arange(T) // 2


def _prep_core_inputs(h, x, z, Wq, bq, Wk, bk, Wv, bv, Wo):
    dsl = slice(h * DH, (h + 1) * DH)

    def dev_w(w, nkc):
        # [nkc*128, DH] -> [p, c, kc*128+m]
        return np.ascontiguousarray(
            w.reshape(nkc, 128, CH, 128).transpose(1, 2, 0, 3)
            .reshape(128, CH, nkc * 128).astype(_bf16))

    wq_h = Wq[:, dsl] * np.float32(0.125)
    bq_h = bq[dsl] * np.float32(0.125)
    wk_h = Wk[:, dsl]
    bk_h = bk[dsl]
    wv_h = Wv[:, dsl][:, _PERM]
    bv_h = bv[dsl][_PERM]
    # Wo rows indexed to match otp: partition p = s + 64*(dt//32),
    # chunk c = dt % 32  ->  head-dim row s*DT + dt
    p_i = np.arange(128)[:, None]
    c_i = np.arange(CH)[None, :]
    ridx = (p_i % 64) * DT + c_i + 32 * (p_i // 64)   # [128, CH]
    wo_h = Wo[dsl, :][ridx]                           # [128, CH, 512]

    zp = z.reshape(T, Z_SIZE)[_TOK]
    xp = x.reshape(T, INPUT_SIZE)[_TOK]
    zt = zp.T.reshape(KCQ, 128, T).transpose(1, 0, 2)
    xt = xp.T.reshape(KCX, 128, T).transpose(1, 0, 2)
    return {
        "zt": np.ascontiguousarray(zt.astype(_bf16)),
        "xt": np.ascontiguousarray(xt.astype(_bf16)),
        "wq": dev_w(wq_h, KCQ),
        "wk": dev_w(wk_h, KCX),
        "wv": dev_w(wv_h, KCX),
        "wo": np.ascontiguousarray(
            wo_h.reshape(128, CH, CT, 128).transpose(0, 2, 1, 3)
            .astype(_bf16)),
        "bq": np.ascontiguousarray(bq_h.reshape(CH, 128).T.astype(np.float32)),
        "bk": np.ascontiguousarray(bk_h.reshape(CH, 128).T.astype(np.float32)),
        "bv": np.ascontiguousarray(bv_h.reshape(CH, 128).T.astype(np.float32)),
    }


def make_in_maps(x, z, Wq, bq, Wk, bk, Wv, bv, Wo):
    x = np.asarray(x, np.float32)
    z = np.asarray(z, np.float32)
    return [
        _prep_core_inputs(h, x, z, np.asarray(Wq, np.float32),
                          np.asarray(bq, np.float32), np.asarray(Wk, np.float32),
                          np.asarray(bk, np.float32), np.asarray(Wv, np.float32),
                          np.asarray(bv, np.float32), np.asarray(Wo, np.float32))
        for h in range(H)
    ]


def get_nc(reps=1):
    key = f"nc{reps}"
    if key not in _cache:
        _cache[key] = _build_nc(reps)
    return _cache[key]


def run_spmd(in_maps, trace=False):
    from concourse.bass_utils import run_bass_kernel_spmd
    nc = get_nc()
    return run_bass_kernel_spmd(nc, in_maps, list(range(H)), trace=trace)


def assemble_output(results, bo):
    total = np.zeros((INPUT_SIZE, T), np.float64)
    for r in results:
        total += r["pt"].astype(np.float64)
    out = total[:, _TPERM].T.astype(np.float32) + np.asarray(bo, np.float32)
    return np.ascontiguousarray(out.reshape(B, N, INPUT_SIZE))


def kernel(x, z, Wq, bq, Wk, bk, Wv, bv, Wo, bo):
    in_maps = make_in_maps(x, z, Wq, bq, Wk, bk, Wv, bv, Wo)
    res = run_spmd(in_maps)
    return assemble_output(res.results, bo)


# revision 47
# speedup vs baseline: 1.3204x; 1.3204x over previous
"""Trainium2 Bass kernel for nn_MultiHeadAttnCoupling.

Reference computation (B=4, N=128, D=32768, heads=8, seq=64, d_tensor=64):
    Q = (z @ Wq + bq).reshape(B,N,H,S,DT)   # per (b,n): attention over S
    K = (x @ Wk + bk).reshape(...)
    V = (x @ Wv + bv).reshape(...)
    out = softmax(Q K^T / 8) V  -> reshape -> @ Wo + bo

Sharding: head-parallel over 8 cores (one head per core); host sums the 8
partial outputs and adds bo.

v2 design ("token pairing"):
  - single 512-token pass; each weight chunk DMA'd once; N=512 proj matmuls.
  - attention processes TWO tokens per matmul via block-diagonal stationaries:
    the 128x128 stationary holds token 2j's K (rows/cols 0-63) and token
    2j+1's K (rows/cols 64-127) with zero off-diagonal blocks (memset once);
    streaming operand stacks the two tokens' q along partitions.  Halves the
    LDWEIGHTS count (the measured bottleneck: ~130ns/LDW regardless of size).
  - all stacked slabs are stored pair-index-innermost so projection evictions
    write contiguous runs (strided writes measured 4-5x slower; strided reads
    are free).  Eviction is 4 ops per chunk (one per src-half x token-parity).
  - attn@V streams V (+ ones column -> softmax denominators in output col 64)
    against the exp'd-scores block-diagonal stationary.
  - output otp is stored in (parity, pair) token order; the host permutes the
    final columns back.
"""

import numpy as np
import ml_dtypes

B, N = 4, 128
INPUT_SIZE, Z_SIZE = 512, 256
DT, H, S = 64, 8, 64
D = DT * H * S            # 32768
DH = S * DT               # 4096 per head
T = B * N                 # 512 tokens
J = T // 2                # 256 token pairs
CH = DH // 128            # 32 chunks per projection
KCQ = Z_SIZE // 128       # 2
KCX = INPUT_SIZE // 128   # 4
CT = INPUT_SIZE // 128    # 4 output col tiles
G = 7                     # pairs per attention group (PSUM bank width)
NW = 3                    # exp'd-scores window ring depth
CB = 4                    # chunks per weight DMA batch
JI = 8                    # kbd pair-block: stationary cols at 16B stride
DB = 6                    # attention groups per otp-rearrange DMA batch

_bf16 = ml_dtypes.bfloat16

_cache = {}


def _build_nc(reps=1):
    import concourse.mybir as mybir
    import concourse.tile as tile
    from concourse import bacc

    f32, bf16 = mybir.dt.float32, mybir.dt.bfloat16
    AF = mybir.ActivationFunctionType
    MUL = mybir.AluOpType.mult

    nc = bacc.Bacc("TRN2", target_bir_lowering=False, debug=False)

    zt_d = nc.dram_tensor("zt", [128, KCQ, T], bf16, kind="ExternalInput")
    xt_d = nc.dram_tensor("xt", [128, KCX, T], bf16, kind="ExternalInput")
    wq_d = nc.dram_tensor("wq", [128, CH, KCQ * 128], bf16, kind="ExternalInput")
    wk_d = nc.dram_tensor("wk", [128, CH, KCX * 128], bf16, kind="ExternalInput")
    wv_d = nc.dram_tensor("wv", [128, CH, KCX * 128], bf16, kind="ExternalInput")
    wo_d = nc.dram_tensor("wo", [128, CT, CH, 128], bf16, kind="ExternalInput")
    bq_d = nc.dram_tensor("bq", [128, CH], f32, kind="ExternalInput")
    bk_d = nc.dram_tensor("bk", [128, CH], f32, kind="ExternalInput")
    bv_d = nc.dram_tensor("bv", [128, CH], f32, kind="ExternalInput")
    pt_d = nc.dram_tensor("pt", [INPUT_SIZE, T], f32, kind="ExternalOutput")

    with tile.TileContext(nc) as tc:
        with (
            tc.tile_pool(name="acts", bufs=1) as acts_pool,
            tc.tile_pool(name="slabs", bufs=1) as slab_pool,
            tc.tile_pool(name="wts", bufs=3) as wts_pool,
            tc.tile_pool(name="rds", bufs=4) as rds_pool,
            tc.tile_pool(name="osb", bufs=2) as osb_pool,
            tc.tile_pool(name="psum", bufs=8, space="PSUM") as psum_pool,
        ):
            # resident activations and biases (z/bq first so Q starts early;
            # x/bk/bv stream in behind the Q projection)
            zt = acts_pool.tile([128, KCQ, T], bf16, tag="zt")
            xt = acts_pool.tile([128, KCX, T], bf16, tag="xt")
            bq = acts_pool.tile([128, CH], f32, tag="bq")
            bk = acts_pool.tile([128, CH], f32, tag="bk")
            bv = acts_pool.tile([128, CH], f32, tag="bv")
            nc.gpsimd.dma_start(zt[:], zt_d[:])
            nc.gpsimd.dma_start(bq[:], bq_d[:])
            nc.gpsimd.dma_start(xt[:], xt_d[:])
            nc.gpsimd.dma_start(bk[:], bk_d[:])
            nc.gpsimd.dma_start(bv[:], bv_d[:])

            for rep in range(reps):
                # stacked slabs, pair index j innermost.
                # kbd[dt+64a, jb, s_k+64a', ji] block-diag (a==a' data, else
                # 0), pair j = jb*JI+ji blocked so LDW cols sit at 16B stride
                # qst[dt+64a, s_q, j]   vst[s_k+64a, dt|ones, j]
                kbd = slab_pool.tile([128, J // JI, 128, JI], bf16, tag="kbd")
                qst = slab_pool.tile([128, S, J], bf16, tag="qst")
                vst = slab_pool.tile([128, DT + 1, J], bf16, tag="vst")
                # exp'd scores ring: ebd[s_k+64a, w, jj, s_q+64a] block-diag
                ebd = slab_pool.tile([128, NW, G, 128], bf16, tag="ebd")
                otp = slab_pool.tile([128, CH, T], bf16, tag="otp")
                # normalized attn@V ring [s_q+64a, dt, ring-pair]; DMA
                # regroups it into otp (partition = s + 64*(dt%2))
                ot2 = slab_pool.tile([128, DT, 2 * DB * G], bf16, tag="ot2")
                # zero the off-diagonal quadrants once (gpsimd is idle)
                nc.gpsimd.memset(kbd[0:64, :, 64:128, :], 0.0)
                nc.gpsimd.memset(kbd[64:128, :, 0:64, :], 0.0)
                nc.gpsimd.memset(ebd[0:64, :, :, 64:128], 0.0)
                nc.gpsimd.memset(ebd[64:128, :, :, 0:64], 0.0)
                nc.vector.memset(vst[:, DT, :], 1.0)

                # ---- projections ----
                # dest quadrant per (src half lo/hi, token parity al):
                #   Q chunk c (s-major): [dt+64al, s_q=2c+hi, j]
                #   K chunk c (s-major): [dt+64al, (s_k=2c+hi)+64al, j]
                #   V chunk c (dt-major): [s_k+64al, dt=2c+hi, j]
                for (w_d, wtag, nkc, act, bias, kind) in (
                    (wq_d, "wq", KCQ, zt, bq, "q"),
                    (wk_d, "wk", KCX, xt, bk, "k"),
                    (wv_d, "wv", KCX, xt, bv, "v"),
                ):
                    for c4 in range(CH // CB):
                        wt = wts_pool.tile([128, CB, nkc, 128], bf16, tag="wt")
                        nc.sync.dma_start(
                            wt[:], w_d[:, CB * c4:CB * (c4 + 1), :].rearrange(
                                "p c (kc m) -> p c kc m", m=128))
                        for ci in range(CB):
                            c = CB * c4 + ci
                            ps = psum_pool.tile([128, T], f32, tag="big",
                                                name=f"pj{rep}{wtag}{c}")
                            for kc in range(nkc):
                                nc.tensor.matmul(
                                    ps[:], wt[:, ci, kc, :], act[:, kc, :],
                                    start=(kc == 0), stop=(kc == nkc - 1))
                            # activations are host-permuted to (parity, pair)
                            # token order: even tokens = cols 0:J, odd = J:2J
                            for hi in range(2):
                                src_lo = ps[64 * hi:64 * hi + 64, 0:J]
                                src_hi = ps[64 * hi:64 * hi + 64, J:T]
                                bia = (None if bias is None else
                                       bias[64 * hi:64 * hi + 64, c:c + 1])
                                m = 2 * c + hi
                                if kind == "q":
                                    # even tokens on ACT, odd tokens on DVE
                                    nc.scalar.activation(
                                        qst[0:64, m, :], src_lo,
                                        AF.Identity, bias=bia)
                                    nc.vector.tensor_scalar_add(
                                        qst[64:128, m, :], src_hi, bia)
                                elif kind == "k":
                                    blk = lambda s: s.rearrange(
                                        "p (a b) -> p a b", b=JI)
                                    nc.scalar.activation(
                                        kbd[0:64, :, m, :], blk(src_lo),
                                        AF.Identity, bias=bia)
                                    nc.vector.tensor_scalar_add(
                                        kbd[64:128, :, 64 + m, :],
                                        blk(src_hi), bia)
                                else:
                                    nc.scalar.activation(
                                        vst[0:64, m, :], src_lo,
                                        AF.Identity, bias=bia)
                                    nc.vector.tensor_scalar_add(
                                        vst[64:128, m, :], src_hi, bia)

                # ---- attention, groups of G pairs ----
                # prefetch all Wo weights now; they land during attention
                # (the Sync queue would otherwise park them behind the last
                # otp drain)
                HC = CH // 2
                wo_tiles = {}
                for ct in range(1):
                    for h2 in range(2):
                        wt = slab_pool.tile([128, HC, 128], bf16,
                                            tag=f"wo{ct}{h2}")
                        nc.sync.dma_start(
                            wt[:], wo_d[:, ct, HC * h2:HC * (h2 + 1), :])
                        wo_tiles[2 * ct + h2] = wt

                # software-pipelined: scores(g+1) issues before attnV(g) so
                # the exp(g) latency on ACT hides under scores(g+1) on PE
                ng = (J + G - 1) // G

                def scores_group(gi):
                    j0 = gi * G
                    g = min(G, J - j0)
                    w = gi % NW
                    sc = psum_pool.tile([128, G, S], f32, tag="big",
                                        name=f"sc{rep}_{gi}")
                    for i in range(g):
                        j = j0 + i
                        nc.tensor.matmul(sc[:, i, :],
                                         kbd[:, j // JI, :, j % JI],
                                         qst[:, :, j],
                                         start=True, stop=True)
                    # exp into block-diag quadrants of the ring window
                    nc.scalar.activation(ebd[0:64, w, 0:g, 0:64],
                                         sc[0:64, 0:g, :], AF.Exp)
                    nc.scalar.activation(ebd[64:128, w, 0:g, 64:128],
                                         sc[64:128, 0:g, :], AF.Exp)

                def attnv_group(gi):
                    j0 = gi * G
                    g = min(G, J - j0)
                    w = gi % NW
                    op = psum_pool.tile([128, G, DT + 1], f32, tag="big",
                                        name=f"op{rep}_{gi}")
                    for i in range(g):
                        nc.tensor.matmul(op[:, i, :], ebd[:, w, i, :],
                                         vst[:, :, j0 + i],
                                         start=True, stop=True)
                    rd = rds_pool.tile([128, G], f32, tag="rd")
                    nc.vector.reciprocal(rd[:, 0:g], op[:, 0:g, DT])
                    # normalize into the ot2 ring (one full-width op)
                    r0 = (gi % (2 * DB)) * G
                    rdb = rd[:, 0:g].unsqueeze(1).broadcast_to([128, DT, g])
                    nc.vector.tensor_tensor(
                        ot2[:, :, r0:r0 + g],
                        op[:, 0:g, 0:DT].transpose([0, 2, 1]), rdb, MUL)

                def drain_batch(b):
                    # DMA ot2 ring half -> otp[s+64*(dt%2), dt//2, al*J+j]
                    g0 = b * DB
                    g1 = min(ng, g0 + DB)
                    jb0, jb1 = g0 * G, min(J, g1 * G)
                    r0 = (g0 % (2 * DB)) * G
                    rn = jb1 - jb0
                    for al in range(2):
                        for dp in range(2):
                            nc.gpsimd.dma_start(
                                otp[64 * dp:64 * dp + 64, :,
                                    al * J + jb0:al * J + jb1],
                                ot2[64 * al:64 * al + 64, dp:DT:2,
                                    r0:r0 + rn])

                nb = (ng + DB - 1) // DB
                for gi in range(ng):
                    scores_group(gi)
                    if gi >= 1:
                        attnv_group(gi - 1)
                        if gi % DB == 0:
                            drain_batch(gi // DB - 1)
                attnv_group(ng - 1)
                for b in range(max(0, (ng - 1) // DB), nb):
                    drain_batch(b)

                # ---- output projection (ct-outer; 2 fins in flight) ----
                for ct in range(CT):
                    fin = psum_pool.tile([128, T], f32, tag="big",
                                         name=f"fin{rep}_{ct}")
                    for h2 in range(2):
                        if 2 * ct + h2 in wo_tiles:
                            wt = wo_tiles[2 * ct + h2]
                        else:
                            wt = wts_pool.tile([128, HC, 128], bf16, tag="wt")
                            nc.sync.dma_start(
                                wt[:], wo_d[:, ct, HC * h2:HC * (h2 + 1), :])
                        for i in range(HC):
                            cc = HC * h2 + i
                            nc.tensor.matmul(
                                fin[:], wt[:, i, :], otp[:, cc, :],
                                start=(cc == 0), stop=(cc == CH - 1))
                    ob = osb_pool.tile([128, T], f32, tag="ob")
                    nc.vector.tensor_copy(ob[:], fin[:])
                    nc.sync.dma_start(pt_d[128 * ct:128 * (ct + 1), :], ob[:])

    nc.compile()
    return nc


# dt-major permutation: new index dt*S+s  <- old index s*DT+dt
_PERM = np.arange(S * DT).reshape(S, DT).T.reshape(-1)
# kernel processes tokens in (parity, pair) order: position t' holds token
# TOK[t'];  _TPERM[t] = position of token t (inverse)
_TOK = np.concatenate([np.arange(0, T, 2), np.arange(1, T, 2)])
_TPERM = (np.arange(T) % 2) * J + np.arange(T) // 2


def _prep_core_inputs(h, x, z, Wq, bq, Wk, bk, Wv, bv, Wo):
    dsl = slice(h * DH, (h + 1) * DH)

    def dev_w(w, nkc):
        # [nkc*128, DH] -> [p, c, kc*128+m]
        return np.ascontiguousarray(
            w.reshape(nkc, 128, CH, 128).transpose(1, 2, 0, 3)
            .reshape(128, CH, nkc * 128).astype(_bf16))

    wq_h = Wq[:, dsl] * np.float32(0.125)
    bq_h = bq[dsl] * np.float32(0.125)
    wk_h = Wk[:, dsl]
    bk_h = bk[dsl]
    wv_h = Wv[:, dsl][:, _PERM]
    bv_h = bv[dsl][_PERM]
    wo_h = Wo[dsl, :][_PERM, :]

    zp = z.reshape(T, Z_SIZE)[_TOK]
    xp = x.reshape(T, INPUT_SIZE)[_TOK]
    zt = zp.T.reshape(KCQ, 128, T).transpose(1, 0, 2)
    xt = xp.T.reshape(KCX, 128, T).transpose(1, 0, 2)
    return {
        "zt": np.ascontiguousarray(zt.astype(_bf16)),
        "xt": np.ascontiguousarray(xt.astype(_bf16)),
        "wq": dev_w(wq_h, KCQ),
        "wk": dev_w(wk_h, KCX),
        "wv": dev_w(wv_h, KCX),
        "wo": np.ascontiguousarray(
            wo_h.reshape(CH, 128, CT, 128).transpose(1, 2, 0, 3)
            .astype(_bf16)),
        "bq": np.ascontiguousarray(bq_h.reshape(CH, 128).T.astype(np.float32)),
        "bk": np.ascontiguousarray(bk_h.reshape(CH, 128).T.astype(np.float32)),
        "bv": np.ascontiguousarray(bv_h.reshape(CH, 128).T.astype(np.float32)),
    }


def make_in_maps(x, z, Wq, bq, Wk, bk, Wv, bv, Wo):
    x = np.asarray(x, np.float32)
    z = np.asarray(z, np.float32)
    return [
        _prep_core_inputs(h, x, z, np.asarray(Wq, np.float32),
                          np.asarray(bq, np.float32), np.asarray(Wk, np.float32),
                          np.asarray(bk, np.float32), np.asarray(Wv, np.float32),
                          np.asarray(bv, np.float32), np.asarray(Wo, np.float32))
        for h in range(H)
    ]


def get_nc(reps=1):
    key = f"nc{reps}"
    if key not in _cache:
        _cache[key] = _build_nc(reps)
    return _cache[key]


def run_spmd(in_maps, trace=False):
    from concourse.bass_utils import run_bass_kernel_spmd
    nc = get_nc()
    return run_bass_kernel_spmd(nc, in_maps, list(range(H)), trace=trace)


def assemble_output(results, bo):
    total = np.zeros((INPUT_SIZE, T), np.float64)
    for r in results:
        total += r["pt"].astype(np.float64)
    out = total[:, _TPERM].T.astype(np.float32) + np.asarray(bo, np.float32)
    return np.ascontiguousarray(out.reshape(B, N, INPUT_SIZE))


def kernel(x, z, Wq, bq, Wk, bk, Wv, bv, Wo, bo):
    in_maps = make_in_maps(x, z, Wq, bq, Wk, bk, Wv, bv, Wo)
    res = run_spmd(in_maps)
    return assemble_output(res.results, bo)


# revision 48
# speedup vs baseline: 1.4845x; 1.1243x over previous
"""Trainium2 Bass kernel for nn_MultiHeadAttnCoupling.

Reference computation (B=4, N=128, D=32768, heads=8, seq=64, d_tensor=64):
    Q = (z @ Wq + bq).reshape(B,N,H,S,DT)   # per (b,n): attention over S
    K = (x @ Wk + bk).reshape(...)
    V = (x @ Wv + bv).reshape(...)
    out = softmax(Q K^T / 8) V  -> reshape -> @ Wo + bo

Sharding: head-parallel over 8 cores (one head per core); host sums the 8
partial outputs and adds bo.

v2 design ("token pairing"):
  - single 512-token pass; each weight chunk DMA'd once; N=512 proj matmuls.
  - attention processes TWO tokens per matmul via block-diagonal stationaries:
    the 128x128 stationary holds token 2j's K (rows/cols 0-63) and token
    2j+1's K (rows/cols 64-127) with zero off-diagonal blocks (memset once);
    streaming operand stacks the two tokens' q along partitions.  Halves the
    LDWEIGHTS count (the measured bottleneck: ~130ns/LDW regardless of size).
  - all stacked slabs are stored pair-index-innermost so projection evictions
    write contiguous runs (strided writes measured 4-5x slower; strided reads
    are free).  Eviction is 4 ops per chunk (one per src-half x token-parity).
  - attn@V streams V (+ ones column -> softmax denominators in output col 64)
    against the exp'd-scores block-diagonal stationary.
  - output otp is stored in (parity, pair) token order; the host permutes the
    final columns back.
"""

import numpy as np
import ml_dtypes

B, N = 4, 128
INPUT_SIZE, Z_SIZE = 512, 256
DT, H, S = 64, 8, 64
D = DT * H * S            # 32768
DH = S * DT               # 4096 per head
T = B * N                 # 512 tokens
J = T // 2                # 256 token pairs
CH = DH // 128            # 32 chunks per projection
KCQ = Z_SIZE // 128       # 2
KCX = INPUT_SIZE // 128   # 4
CT = INPUT_SIZE // 128    # 4 output col tiles
G = 7                     # pairs per attention group (PSUM bank width)
NW = 3                    # exp'd-scores window ring depth
CB = 4                    # chunks per weight DMA batch
JI = 8                    # kbd pair-block: stationary cols at 16B stride
DB = 6                    # attention groups per otp-rearrange DMA batch

_bf16 = ml_dtypes.bfloat16

_cache = {}


def _build_nc(reps=1):
    import concourse.mybir as mybir
    import concourse.tile as tile
    from concourse import bacc

    f32, bf16 = mybir.dt.float32, mybir.dt.bfloat16
    AF = mybir.ActivationFunctionType
    MUL = mybir.AluOpType.mult

    nc = bacc.Bacc("TRN2", target_bir_lowering=False, debug=False)

    zt_d = nc.dram_tensor("zt", [128, KCQ, T], bf16, kind="ExternalInput")
    xt_d = nc.dram_tensor("xt", [128, KCX, T], bf16, kind="ExternalInput")
    wq_d = nc.dram_tensor("wq", [128, CH, KCQ * 128], bf16, kind="ExternalInput")
    wk_d = nc.dram_tensor("wk", [128, CH, KCX * 128], bf16, kind="ExternalInput")
    wv_d = nc.dram_tensor("wv", [128, CH, KCX * 128], bf16, kind="ExternalInput")
    wo_d = nc.dram_tensor("wo", [128, CT, CH, 128], bf16, kind="ExternalInput")
    bq_d = nc.dram_tensor("bq", [128, CH], f32, kind="ExternalInput")
    bk_d = nc.dram_tensor("bk", [128, CH], f32, kind="ExternalInput")
    bv_d = nc.dram_tensor("bv", [128, CH], f32, kind="ExternalInput")
    pt_d = nc.dram_tensor("pt", [INPUT_SIZE, T], f32, kind="ExternalOutput")

    with tile.TileContext(nc) as tc:
        with (
            tc.tile_pool(name="acts", bufs=1) as acts_pool,
            tc.tile_pool(name="slabs", bufs=1) as slab_pool,
            tc.tile_pool(name="wts", bufs=3) as wts_pool,
            tc.tile_pool(name="rds", bufs=4) as rds_pool,
            tc.tile_pool(name="osb", bufs=2) as osb_pool,
            tc.tile_pool(name="psum", bufs=8, space="PSUM") as psum_pool,
        ):
            # resident activations and biases (z/bq first so Q starts early;
            # x/bk/bv stream in behind the Q projection)
            zt = acts_pool.tile([128, KCQ, T], bf16, tag="zt")
            xt = acts_pool.tile([128, KCX, T], bf16, tag="xt")
            bq = acts_pool.tile([128, CH], f32, tag="bq")
            bk = acts_pool.tile([128, CH], f32, tag="bk")
            bv = acts_pool.tile([128, CH], f32, tag="bv")
            nc.gpsimd.dma_start(zt[:], zt_d[:])
            nc.gpsimd.dma_start(bq[:], bq_d[:])
            nc.gpsimd.dma_start(xt[:], xt_d[:])
            nc.gpsimd.dma_start(bk[:], bk_d[:])
            nc.gpsimd.dma_start(bv[:], bv_d[:])

            for rep in range(reps):
                # stacked slabs, pair index j innermost.
                # kbd[dt+64a, jb, s_k+64a', ji] block-diag (a==a' data, else
                # 0), pair j = jb*JI+ji blocked so LDW cols sit at 16B stride
                # qst[dt+64a, s_q, j]   vst[s_k+64a, dt|ones, j]
                kbd = slab_pool.tile([128, J // JI, 128, JI], bf16, tag="kbd")
                qst = slab_pool.tile([128, S, J], bf16, tag="qst")
                vst = slab_pool.tile([128, DT + 1, J], bf16, tag="vst")
                # exp'd scores ring: ebd[s_k+64a, w, jj, s_q+64a] block-diag
                ebd = slab_pool.tile([128, NW, G, 128], bf16, tag="ebd")
                otp = slab_pool.tile([128, CH, T], bf16, tag="otp")
                # normalized attn@V ring [s_q+64a, dt, ring-pair]; DMA
                # regroups it into otp (partition = s + 64*(dt%2))
                ot2 = slab_pool.tile([128, DT, 2 * DB * G], bf16, tag="ot2")
                # zero the off-diagonal quadrants once (gpsimd is idle)
                nc.gpsimd.memset(kbd[0:64, :, 64:128, :], 0.0)
                nc.gpsimd.memset(kbd[64:128, :, 0:64, :], 0.0)
                nc.gpsimd.memset(ebd[0:64, :, :, 64:128], 0.0)
                nc.gpsimd.memset(ebd[64:128, :, :, 0:64], 0.0)
                nc.vector.memset(vst[:, DT, :], 1.0)

                # ---- projections ----
                # dest quadrant per (src half lo/hi, token parity al):
                #   Q chunk c (s-major): [dt+64al, s_q=2c+hi, j]
                #   K chunk c (s-major): [dt+64al, (s_k=2c+hi)+64al, j]
                #   V chunk c (dt-major): [s_k+64al, dt=2c+hi, j]
                for (w_d, wtag, nkc, act, bias, kind) in (
                    (wq_d, "wq", KCQ, zt, bq, "q"),
                    (wk_d, "wk", KCX, xt, bk, "k"),
                    (wv_d, "wv", KCX, xt, bv, "v"),
                ):
                    for c4 in range(CH // CB):
                        wt = wts_pool.tile([128, CB, nkc, 128], bf16, tag="wt")
                        nc.sync.dma_start(
                            wt[:], w_d[:, CB * c4:CB * (c4 + 1), :].rearrange(
                                "p c (kc m) -> p c kc m", m=128))
                        for ci in range(CB):
                            c = CB * c4 + ci
                            ps = psum_pool.tile([128, T], f32, tag="big",
                                                name=f"pj{rep}{wtag}{c}")
                            for kc in range(nkc):
                                nc.tensor.matmul(
                                    ps[:], wt[:, ci, kc, :], act[:, kc, :],
                                    start=(kc == 0), stop=(kc == nkc - 1))
                            # activations are host-permuted to (parity, pair)
                            # token order: even tokens = cols 0:J, odd = J:2J
                            for hi in range(2):
                                src_lo = ps[64 * hi:64 * hi + 64, 0:J]
                                src_hi = ps[64 * hi:64 * hi + 64, J:T]
                                bia = (None if bias is None else
                                       bias[64 * hi:64 * hi + 64, c:c + 1])
                                m = 2 * c + hi
                                if kind == "q":
                                    # even tokens on ACT, odd tokens on DVE
                                    nc.scalar.activation(
                                        qst[0:64, m, :], src_lo,
                                        AF.Identity, bias=bia)
                                    nc.vector.tensor_scalar_add(
                                        qst[64:128, m, :], src_hi, bia)
                                elif kind == "k":
                                    blk = lambda s: s.rearrange(
                                        "p (a b) -> p a b", b=JI)
                                    nc.scalar.activation(
                                        kbd[0:64, :, m, :], blk(src_lo),
                                        AF.Identity, bias=bia)
                                    nc.vector.tensor_scalar_add(
                                        kbd[64:128, :, 64 + m, :],
                                        blk(src_hi), bia)
                                else:
                                    nc.scalar.activation(
                                        vst[0:64, m, :], src_lo,
                                        AF.Identity, bias=bia)
                                    nc.vector.tensor_scalar_add(
                                        vst[64:128, m, :], src_hi, bia)

                # ---- attention, groups of G pairs ----
                # prefetch all Wo weights now; they land during attention
                # (the Sync queue would otherwise park them behind the last
                # otp drain)
                HC = CH // 2
                wo_tiles = {}
                for ct in range(1):
                    for h2 in range(2):
                        wt = slab_pool.tile([128, HC, 128], bf16,
                                            tag=f"wo{ct}{h2}")
                        nc.sync.dma_start(
                            wt[:], wo_d[:, ct, HC * h2:HC * (h2 + 1), :])
                        wo_tiles[2 * ct + h2] = wt

                # software-pipelined: scores(g+1) issues before attnV(g) so
                # the exp(g) latency on ACT hides under scores(g+1) on PE
                ng = (J + G - 1) // G

                def scores_group(gi):
                    j0 = gi * G
                    g = min(G, J - j0)
                    w = gi % NW
                    sc = psum_pool.tile([128, G, S], f32, tag="big",
                                        name=f"sc{rep}_{gi}")
                    for i in range(g):
                        j = j0 + i
                        nc.tensor.matmul(sc[:, i, :],
                                         kbd[:, j // JI, :, j % JI],
                                         qst[:, :, j],
                                         start=True, stop=True)
                    # exp into block-diag quadrants of the ring window
                    nc.scalar.activation(ebd[0:64, w, 0:g, 0:64],
                                         sc[0:64, 0:g, :], AF.Exp)
                    nc.scalar.activation(ebd[64:128, w, 0:g, 64:128],
                                         sc[64:128, 0:g, :], AF.Exp)

                def attnv_group(gi):
                    j0 = gi * G
                    g = min(G, J - j0)
                    w = gi % NW
                    op = psum_pool.tile([128, G, DT + 1], f32, tag="big",
                                        name=f"op{rep}_{gi}")
                    for i in range(g):
                        nc.tensor.matmul(op[:, i, :], ebd[:, w, i, :],
                                         vst[:, :, j0 + i],
                                         start=True, stop=True)
                    rd = rds_pool.tile([128, G], f32, tag="rd")
                    nc.vector.reciprocal(rd[:, 0:g], op[:, 0:g, DT])
                    # normalize into the ot2 ring (one full-width op)
                    r0 = (gi % (2 * DB)) * G
                    rdb = rd[:, 0:g].unsqueeze(1).broadcast_to([128, DT, g])
                    nc.vector.tensor_tensor(
                        ot2[:, :, r0:r0 + g],
                        op[:, 0:g, 0:DT].transpose([0, 2, 1]), rdb, MUL)

                def drain_batch(b):
                    # DMA ot2 ring half -> otp[s+64*(dt%2), dt//2, al*J+j]
                    g0 = b * DB
                    g1 = min(ng, g0 + DB)
                    jb0, jb1 = g0 * G, min(J, g1 * G)
                    r0 = (g0 % (2 * DB)) * G
                    rn = jb1 - jb0
                    for al in range(2):
                        for dp in range(2):
                            nc.gpsimd.dma_start(
                                otp[64 * dp:64 * dp + 64, :,
                                    al * J + jb0:al * J + jb1],
                                ot2[64 * al:64 * al + 64, dp:DT:2,
                                    r0:r0 + rn])

                nb = (ng + DB - 1) // DB
                drained = 0
                for gi in range(ng):
                    scores_group(gi)
                    if gi >= 2:
                        attnv_group(gi - 2)
                        if (gi - 1) % DB == 0 and (gi - 1) // DB >= 1:
                            drain_batch((gi - 1) // DB - 1)
                            drained = (gi - 1) // DB
                attnv_group(ng - 2)
                attnv_group(ng - 1)
                # dense PE warm-up burst: ~3.5us of back-to-back N=512
                # matmuls while the attention tail (exp/TT/drain) finishes,
                # so the Wo phase starts at HAM 8/8 instead of half clock
                heat = psum_pool.tile([128, T], f32, tag="big", name="heat")
                for _ in range(16):
                    nc.tensor.matmul(heat[:], kbd[:, 0, :, 0], xt[:, 0, :],
                                     start=True, stop=True)
                for b in range(drained, nb):
                    drain_batch(b)

                # ---- output projection (ct-outer; 2 fins in flight) ----
                for ct in range(CT):
                    fin = psum_pool.tile([128, T], f32, tag="big",
                                         name=f"fin{rep}_{ct}")
                    for h2 in range(2):
                        if 2 * ct + h2 in wo_tiles:
                            wt = wo_tiles[2 * ct + h2]
                        else:
                            wt = wts_pool.tile([128, HC, 128], bf16, tag="wt")
                            nc.sync.dma_start(
                                wt[:], wo_d[:, ct, HC * h2:HC * (h2 + 1), :])
                        for i in range(HC):
                            cc = HC * h2 + i
                            nc.tensor.matmul(
                                fin[:], wt[:, i, :], otp[:, cc, :],
                                start=(cc == 0), stop=(cc == CH - 1))
                    ob = osb_pool.tile([128, T], f32, tag="ob")
                    nc.vector.tensor_copy(ob[:], fin[:])
                    nc.sync.dma_start(pt_d[128 * ct:128 * (ct + 1), :], ob[:])

    nc.compile()
    return nc


# dt-major permutation: new index dt*S+s  <- old index s*DT+dt
_PERM = np.arange(S * DT).reshape(S, DT).T.reshape(-1)
# kernel processes tokens in (parity, pair) order: position t' holds token
# TOK[t'];  _TPERM[t] = position of token t (inverse)
_TOK = np.concatenate([np.arange(0, T, 2), np.arange(1, T, 2)])
_TPERM = (np.arange(T) % 2) * J + np.arange(T) // 2


def _prep_core_inputs(h, x, z, Wq, bq, Wk, bk, Wv, bv, Wo):
    dsl = slice(h * DH, (h + 1) * DH)

    def dev_w(w, nkc):
        # [nkc*128, DH] -> [p, c, kc*128+m]
        return np.ascontiguousarray(
            w.reshape(nkc, 128, CH, 128).transpose(1, 2, 0, 3)
            .reshape(128, CH, nkc * 128).astype(_bf16))

    wq_h = Wq[:, dsl] * np.float32(0.125)
    bq_h = bq[dsl] * np.float32(0.125)
    wk_h = Wk[:, dsl]
    bk_h = bk[dsl]
    wv_h = Wv[:, dsl][:, _PERM]
    bv_h = bv[dsl][_PERM]
    wo_h = Wo[dsl, :][_PERM, :]

    zp = z.reshape(T, Z_SIZE)[_TOK]
    xp = x.reshape(T, INPUT_SIZE)[_TOK]
    zt = zp.T.reshape(KCQ, 128, T).transpose(1, 0, 2)
    xt = xp.T.reshape(KCX, 128, T).transpose(1, 0, 2)
    return {
        "zt": np.ascontiguousarray(zt.astype(_bf16)),
        "xt": np.ascontiguousarray(xt.astype(_bf16)),
        "wq": dev_w(wq_h, KCQ),
        "wk": dev_w(wk_h, KCX),
        "wv": dev_w(wv_h, KCX),
        "wo": np.ascontiguousarray(
            wo_h.reshape(CH, 128, CT, 128).transpose(1, 2, 0, 3)
            .astype(_bf16)),
        "bq": np.ascontiguousarray(bq_h.reshape(CH, 128).T.astype(np.float32)),
        "bk": np.ascontiguousarray(bk_h.reshape(CH, 128).T.astype(np.float32)),
        "bv": np.ascontiguousarray(bv_h.reshape(CH, 128).T.astype(np.float32)),
    }


def make_in_maps(x, z, Wq, bq, Wk, bk, Wv, bv, Wo):
    x = np.asarray(x, np.float32)
    z = np.asarray(z, np.float32)
    return [
        _prep_core_inputs(h, x, z, np.asarray(Wq, np.float32),
                          np.asarray(bq, np.float32), np.asarray(Wk, np.float32),
                          np.asarray(bk, np.float32), np.asarray(Wv, np.float32),
                          np.asarray(bv, np.float32), np.asarray(Wo, np.float32))
        for h in range(H)
    ]


def get_nc(reps=1):
    key = f"nc{reps}"
    if key not in _cache:
        _cache[key] = _build_nc(reps)
    return _cache[key]


def run_spmd(in_maps, trace=False):
    from concourse.bass_utils import run_bass_kernel_spmd
    nc = get_nc()
    return run_bass_kernel_spmd(nc, in_maps, list(range(H)), trace=trace)


def assemble_output(results, bo):
    total = np.zeros((INPUT_SIZE, T), np.float64)
    for r in results:
        total += r["pt"].astype(np.float64)
    out = total[:, _TPERM].T.astype(np.float32) + np.asarray(bo, np.float32)
    return np.ascontiguousarray(out.reshape(B, N, INPUT_SIZE))


def kernel(x, z, Wq, bq, Wk, bk, Wv, bv, Wo, bo):
    in_maps = make_in_maps(x, z, Wq, bq, Wk, bk, Wv, bv, Wo)
    res = run_spmd(in_maps)
    return assemble_output(res.results, bo)


# revision 49
# speedup vs baseline: 1.6005x; 1.0782x over previous
"""Trainium2 Bass kernel for nn_MultiHeadAttnCoupling.

Reference computation (B=4, N=128, D=32768, heads=8, seq=64, d_tensor=64):
    Q = (z @ Wq + bq).reshape(B,N,H,S,DT)   # per (b,n): attention over S
    K = (x @ Wk + bk).reshape(...)
    V = (x @ Wv + bv).reshape(...)
    out = softmax(Q K^T / 8) V  -> reshape -> @ Wo + bo

Sharding: head-parallel over 8 cores (one head per core); host sums the 8
partial outputs and adds bo.

v2 design ("token pairing"):
  - single 512-token pass; each weight chunk DMA'd once; N=512 proj matmuls.
  - attention processes TWO tokens per matmul via block-diagonal stationaries:
    the 128x128 stationary holds token 2j's K (rows/cols 0-63) and token
    2j+1's K (rows/cols 64-127) with zero off-diagonal blocks (memset once);
    streaming operand stacks the two tokens' q along partitions.  Halves the
    LDWEIGHTS count (the measured bottleneck: ~130ns/LDW regardless of size).
  - all stacked slabs are stored pair-index-innermost so projection evictions
    write contiguous runs (strided writes measured 4-5x slower; strided reads
    are free).  Eviction is 4 ops per chunk (one per src-half x token-parity).
  - attn@V streams V (+ ones column -> softmax denominators in output col 64)
    against the exp'd-scores block-diagonal stationary.
  - output otp is stored in (parity, pair) token order; the host permutes the
    final columns back.
"""

import numpy as np
import ml_dtypes

B, N = 4, 128
INPUT_SIZE, Z_SIZE = 512, 256
DT, H, S = 64, 8, 64
D = DT * H * S            # 32768
DH = S * DT               # 4096 per head
T = B * N                 # 512 tokens
J = T // 2                # 256 token pairs
CH = DH // 128            # 32 chunks per projection
KCQ = Z_SIZE // 128       # 2
KCX = INPUT_SIZE // 128   # 4
CT = INPUT_SIZE // 128    # 4 output col tiles
G = 7                     # pairs per attention group (PSUM bank width)
NW = 3                    # exp'd-scores window ring depth
CB = 4                    # chunks per weight DMA batch
JI = 8                    # kbd pair-block: stationary cols at 16B stride
DB = 6                    # attention groups per otp-rearrange DMA batch

_bf16 = ml_dtypes.bfloat16

_cache = {}


def _build_nc(reps=1):
    import concourse.mybir as mybir
    import concourse.tile as tile
    from concourse import bacc

    f32, bf16 = mybir.dt.float32, mybir.dt.bfloat16
    AF = mybir.ActivationFunctionType
    MUL = mybir.AluOpType.mult

    nc = bacc.Bacc("TRN2", target_bir_lowering=False, debug=False)

    zt_d = nc.dram_tensor("zt", [128, KCQ, T], bf16, kind="ExternalInput")
    xt_d = nc.dram_tensor("xt", [128, KCX, T], bf16, kind="ExternalInput")
    wq_d = nc.dram_tensor("wq", [128, CH, KCQ * 128], bf16, kind="ExternalInput")
    wk_d = nc.dram_tensor("wk", [128, CH, KCX * 128], bf16, kind="ExternalInput")
    wv_d = nc.dram_tensor("wv", [128, CH, KCX * 128], bf16, kind="ExternalInput")
    wo_d = nc.dram_tensor("wo", [128, CT, CH, 128], bf16, kind="ExternalInput")
    bq_d = nc.dram_tensor("bq", [128, CH], f32, kind="ExternalInput")
    bk_d = nc.dram_tensor("bk", [128, CH], f32, kind="ExternalInput")
    bv_d = nc.dram_tensor("bv", [128, CH], f32, kind="ExternalInput")
    pt_d = nc.dram_tensor("pt", [INPUT_SIZE, T], f32, kind="ExternalOutput")

    with tile.TileContext(nc) as tc:
        with (
            tc.tile_pool(name="acts", bufs=1) as acts_pool,
            tc.tile_pool(name="slabs", bufs=1) as slab_pool,
            tc.tile_pool(name="wts", bufs=3) as wts_pool,
            tc.tile_pool(name="rds", bufs=4) as rds_pool,
            tc.tile_pool(name="osb", bufs=2) as osb_pool,
            tc.tile_pool(name="psum", bufs=8, space="PSUM") as psum_pool,
        ):
            # resident activations and biases (z/bq first so Q starts early;
            # x/bk/bv stream in behind the Q projection)
            zt = acts_pool.tile([128, KCQ, T], bf16, tag="zt")
            xt = acts_pool.tile([128, KCX, T], bf16, tag="xt")
            bq = acts_pool.tile([128, CH], f32, tag="bq")
            bk = acts_pool.tile([128, CH], f32, tag="bk")
            bv = acts_pool.tile([128, CH], f32, tag="bv")
            nc.gpsimd.dma_start(zt[:], zt_d[:])
            nc.gpsimd.dma_start(bq[:], bq_d[:])
            nc.gpsimd.dma_start(xt[:], xt_d[:])
            nc.gpsimd.dma_start(bk[:], bk_d[:])
            nc.gpsimd.dma_start(bv[:], bv_d[:])

            for rep in range(reps):
                # stacked slabs, pair index j innermost.
                # kbd[dt+64a, jb, s_k+64a', ji] block-diag (a==a' data, else
                # 0), pair j = jb*JI+ji blocked so LDW cols sit at 16B stride
                # qst[dt+64a, s_q, j]   vst[s_k+64a, dt|ones, j]
                kbd = slab_pool.tile([128, J // JI, 128, JI], bf16, tag="kbd")
                qst = slab_pool.tile([128, S, J], bf16, tag="qst")
                vst = slab_pool.tile([128, DT + 1, J], bf16, tag="vst")
                # exp'd scores ring: ebd[s_k+64a, w, jj, s_q+64a] block-diag
                ebd = slab_pool.tile([128, NW, G, 128], bf16, tag="ebd")
                otp = slab_pool.tile([128, CH, T], bf16, tag="otp")
                # normalized attn@V ring [s_q+64a, dt, ring-pair]; DMA
                # regroups it into otp (partition = s + 64*(dt%2))
                ot2 = slab_pool.tile([128, DT, 2 * DB * G], bf16, tag="ot2")
                # zero the off-diagonal quadrants once (gpsimd is idle)
                nc.gpsimd.memset(kbd[0:64, :, 64:128, :], 0.0)
                nc.gpsimd.memset(kbd[64:128, :, 0:64, :], 0.0)
                nc.gpsimd.memset(ebd[0:64, :, :, 64:128], 0.0)
                nc.gpsimd.memset(ebd[64:128, :, :, 0:64], 0.0)
                nc.vector.memset(vst[:, DT, :], 1.0)

                # ---- projections ----
                # dest quadrant per (src half lo/hi, token parity al):
                #   Q chunk c (s-major): [dt+64al, s_q=2c+hi, j]
                #   K chunk c (s-major): [dt+64al, (s_k=2c+hi)+64al, j]
                #   V chunk c (dt-major): [s_k+64al, dt=2c+hi, j]
                for (w_d, wtag, nkc, act, bias, kind) in (
                    (wq_d, "wq", KCQ, zt, bq, "q"),
                    (wk_d, "wk", KCX, xt, bk, "k"),
                    (wv_d, "wv", KCX, xt, bv, "v"),
                ):
                    for c4 in range(CH // CB):
                        wt = wts_pool.tile([128, CB, nkc, 128], bf16, tag="wt")
                        nc.sync.dma_start(
                            wt[:], w_d[:, CB * c4:CB * (c4 + 1), :].rearrange(
                                "p c (kc m) -> p c kc m", m=128))
                        for ci in range(CB):
                            c = CB * c4 + ci
                            ps = psum_pool.tile([128, T], f32, tag="big",
                                                name=f"pj{rep}{wtag}{c}")
                            for kc in range(nkc):
                                nc.tensor.matmul(
                                    ps[:], wt[:, ci, kc, :], act[:, kc, :],
                                    start=(kc == 0), stop=(kc == nkc - 1))
                            # activations are host-permuted to (parity, pair)
                            # token order: even tokens = cols 0:J, odd = J:2J
                            for hi in range(2):
                                src_lo = ps[64 * hi:64 * hi + 64, 0:J]
                                src_hi = ps[64 * hi:64 * hi + 64, J:T]
                                bia = (None if bias is None else
                                       bias[64 * hi:64 * hi + 64, c:c + 1])
                                m = 2 * c + hi
                                if kind == "q":
                                    # even tokens on ACT, odd tokens on DVE
                                    nc.scalar.activation(
                                        qst[0:64, m, :], src_lo,
                                        AF.Identity, bias=bia)
                                    nc.vector.tensor_scalar_add(
                                        qst[64:128, m, :], src_hi, bia)
                                elif kind == "k":
                                    blk = lambda s: s.rearrange(
                                        "p (a b) -> p a b", b=JI)
                                    nc.scalar.activation(
                                        kbd[0:64, :, m, :], blk(src_lo),
                                        AF.Identity, bias=bia)
                                    nc.vector.tensor_scalar_add(
                                        kbd[64:128, :, 64 + m, :],
                                        blk(src_hi), bia)
                                else:
                                    nc.scalar.activation(
                                        vst[0:64, m, :], src_lo,
                                        AF.Identity, bias=bia)
                                    nc.vector.tensor_scalar_add(
                                        vst[64:128, m, :], src_hi, bia)

                # ---- attention, groups of G pairs ----
                # prefetch all Wo weights now; they land during attention
                # (the Sync queue would otherwise park them behind the last
                # otp drain)
                HC = CH // 2
                wo_tiles = {}
                for ct in range(1):
                    for h2 in range(2):
                        wt = slab_pool.tile([128, HC, 128], bf16,
                                            tag=f"wo{ct}{h2}")
                        nc.sync.dma_start(
                            wt[:], wo_d[:, ct, HC * h2:HC * (h2 + 1), :])
                        wo_tiles[2 * ct + h2] = wt

                # software-pipelined: scores issue ahead of attnV so the
                # exp latency on ACT hides under the next group's scores
                def scores_group(gi):
                    j0 = gi * G
                    g = min(G, J - j0)
                    w = gi % NW
                    sc = psum_pool.tile([128, G, S], f32, tag="big",
                                        name=f"sc{rep}_{gi}")
                    for i in range(g):
                        j = j0 + i
                        nc.tensor.matmul(sc[:, i, :],
                                         kbd[:, j // JI, :, j % JI],
                                         qst[:, :, j],
                                         start=True, stop=True)
                    # exp into block-diag quadrants of the ring window
                    nc.scalar.activation(ebd[0:64, w, 0:g, 0:64],
                                         sc[0:64, 0:g, :], AF.Exp)
                    nc.scalar.activation(ebd[64:128, w, 0:g, 64:128],
                                         sc[64:128, 0:g, :], AF.Exp)

                ng = (J + G - 1) // G
                nfinal = ((ng - 1) // DB - 1) * DB  # groups >= this skip
                                                    # the ring (direct otp)

                def attnv_group(gi):
                    j0 = gi * G
                    g = min(G, J - j0)
                    w = gi % NW
                    op = psum_pool.tile([128, G, DT + 1], f32, tag="big",
                                        name=f"op{rep}_{gi}")
                    for i in range(g):
                        nc.tensor.matmul(op[:, i, :], ebd[:, w, i, :],
                                         vst[:, :, j0 + i],
                                         start=True, stop=True)
                    rd = rds_pool.tile([128, G], f32, tag="rd")
                    nc.vector.reciprocal(rd[:, 0:g], op[:, 0:g, DT])
                    if gi >= nfinal:
                        # tail groups: normalize straight into otp (DVE is
                        # idle here and Wo shouldn't wait on drain DMAs)
                        for al in range(2):
                            pr = slice(64 * al, 64 * al + 64)
                            rdb = rd[pr, 0:g].unsqueeze(1).broadcast_to(
                                [64, CH, g])
                            tsl = slice(al * J + j0, al * J + j0 + g)
                            nc.vector.tensor_tensor(
                                otp[0:64, :, tsl],
                                op[pr, 0:g, 0:DT:2].transpose([0, 2, 1]),
                                rdb, MUL)
                            nc.vector.tensor_tensor(
                                otp[64:128, :, tsl],
                                op[pr, 0:g, 1:DT:2].transpose([0, 2, 1]),
                                rdb, MUL)
                        return
                    # normalize into the ot2 ring (one full-width op)
                    r0 = (gi % (2 * DB)) * G
                    rdb = rd[:, 0:g].unsqueeze(1).broadcast_to([128, DT, g])
                    nc.vector.tensor_tensor(
                        ot2[:, :, r0:r0 + g],
                        op[:, 0:g, 0:DT].transpose([0, 2, 1]), rdb, MUL)

                def drain_batch(b):
                    # DMA ot2 ring half -> otp[s+64*(dt%2), dt//2, al*J+j]
                    g0 = b * DB
                    g1 = min(ng, g0 + DB)
                    jb0, jb1 = g0 * G, min(J, g1 * G)
                    r0 = (g0 % (2 * DB)) * G
                    rn = jb1 - jb0
                    for al in range(2):
                        for dp in range(2):
                            nc.gpsimd.dma_start(
                                otp[64 * dp:64 * dp + 64, :,
                                    al * J + jb0:al * J + jb1],
                                ot2[64 * al:64 * al + 64, dp:DT:2,
                                    r0:r0 + rn])

                drained = 0
                for gi in range(ng):
                    scores_group(gi)
                    if gi >= 2:
                        attnv_group(gi - 2)
                        if (gi - 1) % DB == 0 and (gi - 1) // DB >= 1:
                            drain_batch((gi - 1) // DB - 1)
                            drained = (gi - 1) // DB
                attnv_group(ng - 2)
                attnv_group(ng - 1)
                # dense PE warm-up burst: ~3.5us of back-to-back N=512
                # matmuls while the attention tail (exp/TT/drain) finishes,
                # so the Wo phase starts at HAM 8/8 instead of half clock
                heat = psum_pool.tile([128, T], f32, tag="big", name="heat")
                for _ in range(16):
                    nc.tensor.matmul(heat[:], kbd[:, 0, :, 0], xt[:, 0, :],
                                     start=True, stop=True)
                for b in range(drained, nfinal // DB):
                    drain_batch(b)

                # ---- output projection (ct-outer; 2 fins in flight) ----
                for ct in range(CT):
                    fin = psum_pool.tile([128, T], f32, tag="big",
                                         name=f"fin{rep}_{ct}")
                    for h2 in range(2):
                        if 2 * ct + h2 in wo_tiles:
                            wt = wo_tiles[2 * ct + h2]
                        else:
                            wt = wts_pool.tile([128, HC, 128], bf16, tag="wt")
                            nc.sync.dma_start(
                                wt[:], wo_d[:, ct, HC * h2:HC * (h2 + 1), :])
                        for i in range(HC):
                            cc = HC * h2 + i
                            nc.tensor.matmul(
                                fin[:], wt[:, i, :], otp[:, cc, :],
                                start=(cc == 0), stop=(cc == CH - 1))
                    ob = osb_pool.tile([128, T], f32, tag="ob")
                    nc.vector.tensor_copy(ob[:], fin[:])
                    nc.sync.dma_start(pt_d[128 * ct:128 * (ct + 1), :], ob[:])

    nc.compile()
    return nc


# dt-major permutation: new index dt*S+s  <- old index s*DT+dt
_PERM = np.arange(S * DT).reshape(S, DT).T.reshape(-1)
# kernel processes tokens in (parity, pair) order: position t' holds token
# TOK[t'];  _TPERM[t] = position of token t (inverse)
_TOK = np.concatenate([np.arange(0, T, 2), np.arange(1, T, 2)])
_TPERM = (np.arange(T) % 2) * J + np.arange(T) // 2


def _prep_core_inputs(h, x, z, Wq, bq, Wk, bk, Wv, bv, Wo):
    dsl = slice(h * DH, (h + 1) * DH)

    def dev_w(w, nkc):
        # [nkc*128, DH] -> [p, c, kc*128+m]
        return np.ascontiguousarray(
            w.reshape(nkc, 128, CH, 128).transpose(1, 2, 0, 3)
            .reshape(128, CH, nkc * 128).astype(_bf16))

    wq_h = Wq[:, dsl] * np.float32(0.125)
    bq_h = bq[dsl] * np.float32(0.125)
    wk_h = Wk[:, dsl]
    bk_h = bk[dsl]
    wv_h = Wv[:, dsl][:, _PERM]
    bv_h = bv[dsl][_PERM]
    wo_h = Wo[dsl, :][_PERM, :]

    zp = z.reshape(T, Z_SIZE)[_TOK]
    xp = x.reshape(T, INPUT_SIZE)[_TOK]
    zt = zp.T.reshape(KCQ, 128, T).transpose(1, 0, 2)
    xt = xp.T.reshape(KCX, 128, T).transpose(1, 0, 2)
    return {
        "zt": np.ascontiguousarray(zt.astype(_bf16)),
        "xt": np.ascontiguousarray(xt.astype(_bf16)),
        "wq": dev_w(wq_h, KCQ),
        "wk": dev_w(wk_h, KCX),
        "wv": dev_w(wv_h, KCX),
        "wo": np.ascontiguousarray(
            wo_h.reshape(CH, 128, CT, 128).transpose(1, 2, 0, 3)
            .astype(_bf16)),
        "bq": np.ascontiguousarray(bq_h.reshape(CH, 128).T.astype(np.float32)),
        "bk": np.ascontiguousarray(bk_h.reshape(CH, 128).T.astype(np.float32)),
        "bv": np.ascontiguousarray(bv_h.reshape(CH, 128).T.astype(np.float32)),
    }


def make_in_maps(x, z, Wq, bq, Wk, bk, Wv, bv, Wo):
    x = np.asarray(x, np.float32)
    z = np.asarray(z, np.float32)
    return [
        _prep_core_inputs(h, x, z, np.asarray(Wq, np.float32),
                          np.asarray(bq, np.float32), np.asarray(Wk, np.float32),
                          np.asarray(bk, np.float32), np.asarray(Wv, np.float32),
                          np.asarray(bv, np.float32), np.asarray(Wo, np.float32))
        for h in range(H)
    ]


def get_nc(reps=1):
    key = f"nc{reps}"
    if key not in _cache:
        _cache[key] = _build_nc(reps)
    return _cache[key]


def run_spmd(in_maps, trace=False):
    from concourse.bass_utils import run_bass_kernel_spmd
    nc = get_nc()
    return run_bass_kernel_spmd(nc, in_maps, list(range(H)), trace=trace)


def assemble_output(results, bo):
    total = np.zeros((INPUT_SIZE, T), np.float64)
    for r in results:
        total += r["pt"].astype(np.float64)
    out = total[:, _TPERM].T.astype(np.float32) + np.asarray(bo, np.float32)
    return np.ascontiguousarray(out.reshape(B, N, INPUT_SIZE))


def kernel(x, z, Wq, bq, Wk, bk, Wv, bv, Wo, bo):
    in_maps = make_in_maps(x, z, Wq, bq, Wk, bk, Wv, bv, Wo)
    res = run_spmd(in_maps)
    return assemble_output(res.results, bo)


# revision 50
# speedup vs baseline: 1.6041x; 1.0023x over previous
"""Trainium2 Bass kernel for nn_MultiHeadAttnCoupling.

Reference computation (B=4, N=128, D=32768, heads=8, seq=64, d_tensor=64):
    Q = (z @ Wq + bq).reshape(B,N,H,S,DT)   # per (b,n): attention over S
    K = (x @ Wk + bk).reshape(...)
    V = (x @ Wv + bv).reshape(...)
    out = softmax(Q K^T / 8) V  -> reshape -> @ Wo + bo

Sharding: head-parallel over 8 cores (one head per core); host sums the 8
partial outputs and adds bo.

Design (measured-driven; ~197us vs 319us for the naive per-token kernel):
  - single 512-token pass; each weight chunk DMA'd once; N=512 proj matmuls.
  - attention processes TWO tokens per matmul via block-diagonal stationaries:
    the 128x128 stationary holds token 2j's K (rows/cols 0-63) and token
    2j+1's K (rows/cols 64-127) with zero off-diagonal blocks (memset once);
    the streaming operand stacks the two tokens' q along partitions.  Halves
    the LDWEIGHTS count (the measured bottleneck: ~117ns/LDW regardless of
    width; strided stationary cols cost 2x unless blocked to >=16B stride).
  - host pre-permutes activations to (parity, pair) token order so the
    projection evictions read/write contiguous runs (strided engine WRITES
    measured 4-5x slower; strided reads are free).  4 eviction ops per chunk
    (src-half x token-parity quadrants), split evenly over ScalarE/VectorE.
  - attn@V streams V (+ ones column -> softmax denominators in output col 64)
    against the exp'd-scores block-diagonal stationary; scores for group g+2
    issue before attn@V of group g so the exp latency hides on the PE queue.
  - normalization is one full-width VectorE op per 7-pair group into a ring;
    idle DMA engines regroup the ring into the Wo-ready otp layout.  The
    final 7 groups normalize straight into otp so Wo never waits on DMA.
  - a dense burst of 16 throwaway N=512 matmuls re-warms the PE clock (HAM
    K=8/8) during the attention tail so the Wo phase runs at 2.4GHz.
  - output otp is in (parity, pair) token order; the host permutes it back.
"""

import numpy as np
import ml_dtypes

B, N = 4, 128
INPUT_SIZE, Z_SIZE = 512, 256
DT, H, S = 64, 8, 64
D = DT * H * S            # 32768
DH = S * DT               # 4096 per head
T = B * N                 # 512 tokens
J = T // 2                # 256 token pairs
CH = DH // 128            # 32 chunks per projection
KCQ = Z_SIZE // 128       # 2
KCX = INPUT_SIZE // 128   # 4
CT = INPUT_SIZE // 128    # 4 output col tiles
G = 7                     # pairs per attention group (PSUM bank width)
NW = 3                    # exp'd-scores window ring depth
CB = 4                    # chunks per weight DMA batch
JI = 8                    # kbd pair-block: stationary cols at 16B stride
DB = 6                    # attention groups per otp-rearrange DMA batch

_bf16 = ml_dtypes.bfloat16

_cache = {}


def _build_nc(reps=1):
    import concourse.mybir as mybir
    import concourse.tile as tile
    from concourse import bacc

    f32, bf16 = mybir.dt.float32, mybir.dt.bfloat16
    AF = mybir.ActivationFunctionType
    MUL = mybir.AluOpType.mult

    nc = bacc.Bacc("TRN2", target_bir_lowering=False, debug=False)

    zt_d = nc.dram_tensor("zt", [128, KCQ, T], bf16, kind="ExternalInput")
    xt_d = nc.dram_tensor("xt", [128, KCX, T], bf16, kind="ExternalInput")
    wq_d = nc.dram_tensor("wq", [128, CH, KCQ * 128], bf16, kind="ExternalInput")
    wk_d = nc.dram_tensor("wk", [128, CH, KCX * 128], bf16, kind="ExternalInput")
    wv_d = nc.dram_tensor("wv", [128, CH, KCX * 128], bf16, kind="ExternalInput")
    wo_d = nc.dram_tensor("wo", [128, CT, CH, 128], bf16, kind="ExternalInput")
    bq_d = nc.dram_tensor("bq", [128, CH], f32, kind="ExternalInput")
    bk_d = nc.dram_tensor("bk", [128, CH], f32, kind="ExternalInput")
    bv_d = nc.dram_tensor("bv", [128, CH], f32, kind="ExternalInput")
    pt_d = nc.dram_tensor("pt", [INPUT_SIZE, T], f32, kind="ExternalOutput")

    with tile.TileContext(nc) as tc:
        with (
            tc.tile_pool(name="acts", bufs=1) as acts_pool,
            tc.tile_pool(name="slabs", bufs=1) as slab_pool,
            tc.tile_pool(name="wts", bufs=3) as wts_pool,
            tc.tile_pool(name="rds", bufs=4) as rds_pool,
            tc.tile_pool(name="osb", bufs=2) as osb_pool,
            tc.tile_pool(name="psum", bufs=8, space="PSUM") as psum_pool,
        ):
            # resident activations and biases (z/bq first so Q starts early;
            # x/bk/bv stream in behind the Q projection)
            zt = acts_pool.tile([128, KCQ, T], bf16, tag="zt")
            xt = acts_pool.tile([128, KCX, T], bf16, tag="xt")
            bq = acts_pool.tile([128, CH], f32, tag="bq")
            bk = acts_pool.tile([128, CH], f32, tag="bk")
            bv = acts_pool.tile([128, CH], f32, tag="bv")
            nc.gpsimd.dma_start(zt[:], zt_d[:])
            nc.gpsimd.dma_start(bq[:], bq_d[:])
            nc.gpsimd.dma_start(xt[:], xt_d[:])
            nc.gpsimd.dma_start(bk[:], bk_d[:])
            nc.gpsimd.dma_start(bv[:], bv_d[:])

            for rep in range(reps):
                # stacked slabs, pair index j innermost.
                # kbd[dt+64a, jb, s_k+64a', ji] block-diag (a==a' data, else
                # 0), pair j = jb*JI+ji blocked so LDW cols sit at 16B stride
                # qst[dt+64a, s_q, j]   vst[s_k+64a, dt|ones, j]
                kbd = slab_pool.tile([128, J // JI, 128, JI], bf16, tag="kbd")
                qst = slab_pool.tile([128, S, J], bf16, tag="qst")
                vst = slab_pool.tile([128, DT + 1, J], bf16, tag="vst")
                # exp'd scores ring: ebd[s_k+64a, w, jj, s_q+64a] block-diag
                ebd = slab_pool.tile([128, NW, G, 128], bf16, tag="ebd")
                otp = slab_pool.tile([128, CH, T], bf16, tag="otp")
                # normalized attn@V ring [s_q+64a, dt, ring-pair]; DMA
                # regroups it into otp (partition = s + 64*(dt%2))
                ot2 = slab_pool.tile([128, DT, 2 * DB * G], bf16, tag="ot2")
                # zero the off-diagonal quadrants once (gpsimd is idle)
                nc.gpsimd.memset(kbd[0:64, :, 64:128, :], 0.0)
                nc.gpsimd.memset(kbd[64:128, :, 0:64, :], 0.0)
                nc.gpsimd.memset(ebd[0:64, :, :, 64:128], 0.0)
                nc.gpsimd.memset(ebd[64:128, :, :, 0:64], 0.0)
                nc.vector.memset(vst[:, DT, :], 1.0)

                # ---- projections ----
                # dest quadrant per (src half lo/hi, token parity al):
                #   Q chunk c (s-major): [dt+64al, s_q=2c+hi, j]
                #   K chunk c (s-major): [dt+64al, (s_k=2c+hi)+64al, j]
                #   V chunk c (dt-major): [s_k+64al, dt=2c+hi, j]
                for (w_d, wtag, nkc, act, bias, kind) in (
                    (wq_d, "wq", KCQ, zt, bq, "q"),
                    (wk_d, "wk", KCX, xt, bk, "k"),
                    (wv_d, "wv", KCX, xt, bv, "v"),
                ):
                    for c4 in range(CH // CB):
                        wt = wts_pool.tile([128, CB, nkc, 128], bf16, tag="wt")
                        nc.sync.dma_start(
                            wt[:], w_d[:, CB * c4:CB * (c4 + 1), :].rearrange(
                                "p c (kc m) -> p c kc m", m=128))
                        for ci in range(CB):
                            c = CB * c4 + ci
                            ps = psum_pool.tile([128, T], f32, tag="big",
                                                name=f"pj{rep}{wtag}{c}")
                            for kc in range(nkc):
                                nc.tensor.matmul(
                                    ps[:], wt[:, ci, kc, :], act[:, kc, :],
                                    start=(kc == 0), stop=(kc == nkc - 1))
                            # activations are host-permuted to (parity, pair)
                            # token order: even tokens = cols 0:J, odd = J:2J
                            for hi in range(2):
                                src_lo = ps[64 * hi:64 * hi + 64, 0:J]
                                src_hi = ps[64 * hi:64 * hi + 64, J:T]
                                bia = (None if bias is None else
                                       bias[64 * hi:64 * hi + 64, c:c + 1])
                                m = 2 * c + hi
                                if kind == "q":
                                    # even tokens on ACT, odd tokens on DVE
                                    nc.scalar.activation(
                                        qst[0:64, m, :], src_lo,
                                        AF.Identity, bias=bia)
                                    nc.vector.tensor_scalar_add(
                                        qst[64:128, m, :], src_hi, bia)
                                elif kind == "k":
                                    blk = lambda s: s.rearrange(
                                        "p (a b) -> p a b", b=JI)
                                    nc.scalar.activation(
                                        kbd[0:64, :, m, :], blk(src_lo),
                                        AF.Identity, bias=bia)
                                    nc.vector.tensor_scalar_add(
                                        kbd[64:128, :, 64 + m, :],
                                        blk(src_hi), bia)
                                else:
                                    nc.scalar.activation(
                                        vst[0:64, m, :], src_lo,
                                        AF.Identity, bias=bia)
                                    nc.vector.tensor_scalar_add(
                                        vst[64:128, m, :], src_hi, bia)

                # ---- attention, groups of G pairs ----
                # prefetch all Wo weights now; they land during attention
                # (the Sync queue would otherwise park them behind the last
                # otp drain)
                HC = CH // 2
                wo_tiles = {}
                for ct in range(1):
                    for h2 in range(2):
                        wt = slab_pool.tile([128, HC, 128], bf16,
                                            tag=f"wo{ct}{h2}")
                        nc.sync.dma_start(
                            wt[:], wo_d[:, ct, HC * h2:HC * (h2 + 1), :])
                        wo_tiles[2 * ct + h2] = wt

                # software-pipelined: scores issue ahead of attnV so the
                # exp latency on ACT hides under the next group's scores
                def scores_group(gi):
                    j0 = gi * G
                    g = min(G, J - j0)
                    w = gi % NW
                    sc = psum_pool.tile([128, G, S], f32, tag="big",
                                        name=f"sc{rep}_{gi}")
                    for i in range(g):
                        j = j0 + i
                        nc.tensor.matmul(sc[:, i, :],
                                         kbd[:, j // JI, :, j % JI],
                                         qst[:, :, j],
                                         start=True, stop=True)
                    # exp into block-diag quadrants of the ring window
                    nc.scalar.activation(ebd[0:64, w, 0:g, 0:64],
                                         sc[0:64, 0:g, :], AF.Exp)
                    nc.scalar.activation(ebd[64:128, w, 0:g, 64:128],
                                         sc[64:128, 0:g, :], AF.Exp)

                ng = (J + G - 1) // G
                nfinal = ((ng - 1) // DB - 1) * DB  # groups >= this skip
                                                    # the ring (direct otp)

                def attnv_group(gi):
                    j0 = gi * G
                    g = min(G, J - j0)
                    w = gi % NW
                    op = psum_pool.tile([128, G, DT + 1], f32, tag="big",
                                        name=f"op{rep}_{gi}")
                    for i in range(g):
                        nc.tensor.matmul(op[:, i, :], ebd[:, w, i, :],
                                         vst[:, :, j0 + i],
                                         start=True, stop=True)
                    rd = rds_pool.tile([128, G], f32, tag="rd")
                    nc.vector.reciprocal(rd[:, 0:g], op[:, 0:g, DT])
                    if gi >= nfinal:
                        # tail groups: normalize straight into otp (DVE is
                        # idle here and Wo shouldn't wait on drain DMAs)
                        for al in range(2):
                            pr = slice(64 * al, 64 * al + 64)
                            rdb = rd[pr, 0:g].unsqueeze(1).broadcast_to(
                                [64, CH, g])
                            tsl = slice(al * J + j0, al * J + j0 + g)
                            nc.vector.tensor_tensor(
                                otp[0:64, :, tsl],
                                op[pr, 0:g, 0:DT:2].transpose([0, 2, 1]),
                                rdb, MUL)
                            nc.vector.tensor_tensor(
                                otp[64:128, :, tsl],
                                op[pr, 0:g, 1:DT:2].transpose([0, 2, 1]),
                                rdb, MUL)
                        return
                    # normalize into the ot2 ring (one full-width op)
                    r0 = (gi % (2 * DB)) * G
                    rdb = rd[:, 0:g].unsqueeze(1).broadcast_to([128, DT, g])
                    nc.vector.tensor_tensor(
                        ot2[:, :, r0:r0 + g],
                        op[:, 0:g, 0:DT].transpose([0, 2, 1]), rdb, MUL)

                def drain_batch(b):
                    # DMA ot2 ring half -> otp[s+64*(dt%2), dt//2, al*J+j]
                    g0 = b * DB
                    g1 = min(ng, g0 + DB)
                    jb0, jb1 = g0 * G, min(J, g1 * G)
                    r0 = (g0 % (2 * DB)) * G
                    rn = jb1 - jb0
                    for al in range(2):
                        for dp in range(2):
                            nc.gpsimd.dma_start(
                                otp[64 * dp:64 * dp + 64, :,
                                    al * J + jb0:al * J + jb1],
                                ot2[64 * al:64 * al + 64, dp:DT:2,
                                    r0:r0 + rn])

                drained = 0
                for gi in range(ng):
                    scores_group(gi)
                    if gi >= 2:
                        attnv_group(gi - 2)
                        if (gi - 1) % DB == 0 and (gi - 1) // DB >= 1:
                            drain_batch((gi - 1) // DB - 1)
                            drained = (gi - 1) // DB
                attnv_group(ng - 2)
                attnv_group(ng - 1)
                # dense PE warm-up burst: ~3.5us of back-to-back N=512
                # matmuls while the attention tail (exp/TT/drain) finishes,
                # so the Wo phase starts at HAM 8/8 instead of half clock
                heat = psum_pool.tile([128, T], f32, tag="big", name="heat")
                for _ in range(16):
                    nc.tensor.matmul(heat[:], kbd[:, 0, :, 0], xt[:, 0, :],
                                     start=True, stop=True)
                for b in range(drained, nfinal // DB):
                    drain_batch(b)

                # ---- output projection (ct-outer; 2 fins in flight) ----
                for ct in range(CT):
                    fin = psum_pool.tile([128, T], f32, tag="big",
                                         name=f"fin{rep}_{ct}")
                    for h2 in range(2):
                        if 2 * ct + h2 in wo_tiles:
                            wt = wo_tiles[2 * ct + h2]
                        else:
                            wt = wts_pool.tile([128, HC, 128], bf16, tag="wt")
                            nc.sync.dma_start(
                                wt[:], wo_d[:, ct, HC * h2:HC * (h2 + 1), :])
                        for i in range(HC):
                            cc = HC * h2 + i
                            nc.tensor.matmul(
                                fin[:], wt[:, i, :], otp[:, cc, :],
                                start=(cc == 0), stop=(cc == CH - 1))
                    ob = osb_pool.tile([128, T], f32, tag="ob")
                    nc.vector.tensor_copy(ob[:], fin[:])
                    nc.sync.dma_start(pt_d[128 * ct:128 * (ct + 1), :], ob[:])

    nc.compile()
    return nc


# dt-major permutation: new index dt*S+s  <- old index s*DT+dt
_PERM = np.arange(S * DT).reshape(S, DT).T.reshape(-1)
# kernel processes tokens in (parity, pair) order: position t' holds token
# TOK[t'];  _TPERM[t] = position of token t (inverse)
_TOK = np.concatenate([np.arange(0, T, 2), np.arange(1, T, 2)])
_TPERM = (np.arange(T) % 2) * J + np.arange(T) // 2


def _prep_core_inputs(h, x, z, Wq, bq, Wk, bk, Wv, bv, Wo):
    dsl = slice(h * DH, (h + 1) * DH)

    def dev_w(w, nkc):
        # [nkc*128, DH] -> [p, c, kc*128+m]
        return np.ascontiguousarray(
            w.reshape(nkc, 128, CH, 128).transpose(1, 2, 0, 3)
            .reshape(128, CH, nkc * 128).astype(_bf16))

    wq_h = Wq[:, dsl] * np.float32(0.125)
    bq_h = bq[dsl] * np.float32(0.125)
    wk_h = Wk[:, dsl]
    bk_h = bk[dsl]
    wv_h = Wv[:, dsl][:, _PERM]
    bv_h = bv[dsl][_PERM]
    wo_h = Wo[dsl, :][_PERM, :]

    zp = z.reshape(T, Z_SIZE)[_TOK]
    xp = x.reshape(T, INPUT_SIZE)[_TOK]
    zt = zp.T.reshape(KCQ, 128, T).transpose(1, 0, 2)
    xt = xp.T.reshape(KCX, 128, T).transpose(1, 0, 2)
    return {
        "zt": np.ascontiguousarray(zt.astype(_bf16)),
        "xt": np.ascontiguousarray(xt.astype(_bf16)),
        "wq": dev_w(wq_h, KCQ),
        "wk": dev_w(wk_h, KCX),
        "wv": dev_w(wv_h, KCX),
        "wo": np.ascontiguousarray(
            wo_h.reshape(CH, 128, CT, 128).transpose(1, 2, 0, 3)
            .astype(_bf16)),
        "bq": np.ascontiguousarray(bq_h.reshape(CH, 128).T.astype(np.float32)),
        "bk": np.ascontiguousarray(bk_h.reshape(CH, 128).T.astype(np.float32)),
        "bv": np.ascontiguousarray(bv_h.reshape(CH, 128).T.astype(np.float32)),
    }


def make_in_maps(x, z, Wq, bq, Wk, bk, Wv, bv, Wo):
    x = np.asarray(x, np.float32)
    z = np.asarray(z, np.float32)
    return [
        _prep_core_inputs(h, x, z, np.asarray(Wq, np.float32),
                          np.asarray(bq, np.float32), np.asarray(Wk, np.float32),
                          np.asarray(bk, np.float32), np.asarray(Wv, np.float32),
                          np.asarray(bv, np.float32), np.asarray(Wo, np.float32))
        for h in range(H)
    ]


def get_nc(reps=1):
    key = f"nc{reps}"
    if key not in _cache:
        _cache[key] = _build_nc(reps)
    return _cache[key]


def run_spmd(in_maps, trace=False):
    from concourse.bass_utils import run_bass_kernel_spmd
    nc = get_nc()
    return run_bass_kernel_spmd(nc, in_maps, list(range(H)), trace=trace)


def assemble_output(results, bo):
    total = np.zeros((INPUT_SIZE, T), np.float64)
    for r in results:
        total += r["pt"].astype(np.float64)
    out = total[:, _TPERM].T.astype(np.float32) + np.asarray(bo, np.float32)
    return np.ascontiguousarray(out.reshape(B, N, INPUT_SIZE))


def kernel(x, z, Wq, bq, Wk, bk, Wv, bv, Wo, bo):
    in_maps = make_in_maps(x, z, Wq, bq, Wk, bk, Wv, bv, Wo)
    res = run_spmd(in_maps)
    return assemble_output(res.results, bo)
